# revision 66
# baseline (speedup 1.0000x reference)
"""Trainium2 Bass kernel for a GPT-style block with sliding-window attention.

Sharding: 8 cores = batch(2) x sequence-quarters(4). Each core processes its
1024 tokens end-to-end (LN1 -> QKV -> windowed attention -> proj -> residual ->
LN2 -> FFN(gelu) -> residual), with a 256-token halo recomputed for K/V.
No collectives. Activations are feature-major (features on partitions, tokens
on the free dim) so every matmul chains directly; output stays feature-major
and is transposed on the host.

v2 (vs baseline): bf16 weights+activations (f32 PSUM/stats), attention band
trimmed to the 1280 valid score columns per 256-query block, softmax
denominator normalized via reciprocal_approx_fast + a K=1 broadcast matmul
(no DRAM bounce), qb-outer attention ordering so c_proj/LN2/FFN overlap, FFN2
accumulated 24-deep in PSUM (no DVE accumulation), and no output transpose.
"""
import contextlib
import numpy as np

import concourse.bass as bass
import concourse.mybir as mybir
import concourse.tile as tile
from concourse import bacc
from concourse.bass_utils import run_bass_kernel_spmd

F32R = mybir.dt.float32r
F32 = mybir.dt.float32
BF16 = mybir.dt.bfloat16
ALU = mybir.AluOpType
ACTF = mybir.ActivationFunctionType

B, S, E, H, D, WIN = 2, 4096, 768, 12, 64, 256
NSEQ = 4                      # sequence shards per batch
CHUNK = S // NSEQ             # 1024 core tokens per core
EXT = CHUNK + 2 * WIN         # 1536 extended tokens (k/v halo)
KC = E // 128                 # 6 chunks of the embedding dim
NT_EXT = EXT // 128           # 12
EPS = 1e-5


def mktile(pool, shape, dtype, tag):
    return pool.tile(shape, dtype, tag=tag, name=tag)


def build():
    nc = bacc.Bacc("TRN2", target_bir_lowering=False, debug=False, num_devices=8)

    g = {}
    g["xT"] = nc.dram_tensor("xT", [E, EXT], BF16, kind="ExternalInput")
    g["w1"] = nc.dram_tensor("w1", [E, 3 * E], BF16, kind="ExternalInput")
    g["b1"] = nc.dram_tensor("b1", [128, 12], F32, kind="ExternalInput")
    g["bv"] = nc.dram_tensor("bv", [1, E], BF16, kind="ExternalInput")
    g["wp"] = nc.dram_tensor("wp", [E, E], BF16, kind="ExternalInput")
    g["bp"] = nc.dram_tensor("bp", [128, KC], F32, kind="ExternalInput")
    g["w3"] = nc.dram_tensor("w3", [E, 4 * E], BF16, kind="ExternalInput")
    g["b3"] = nc.dram_tensor("b3", [128, 24], F32, kind="ExternalInput")
    g["w4"] = nc.dram_tensor("w4", [4 * E, E], BF16, kind="ExternalInput")
    g["b4"] = nc.dram_tensor("b4", [128, KC], F32, kind="ExternalInput")
    g["m01"] = nc.dram_tensor("m01", [4, 128, 384], BF16, kind="ExternalInput")
    g["m45"] = nc.dram_tensor("m45", [4, 128, 384], BF16, kind="ExternalInput")
    g["ones"] = nc.dram_tensor("ones", [128, 128], BF16, kind="ExternalInput")
    g["ones_f"] = nc.dram_tensor("ones_f", [1, 128], F32, kind="ExternalInput")
    g["out"] = nc.dram_tensor("out", [E, CHUNK], BF16, kind="ExternalOutput")

    with tile.TileContext(nc) as tc:
        with tc.tile_pool(name="const", bufs=1) as const:
            g["ones128"] = mktile(const, [128, 128], BF16, "ones128")
            nc.sync.dma_start(out=g["ones128"], in_=g["ones"].ap())
            g["ones_row"] = mktile(const, [1, 128], BF16, "ones_row")
            nc.sync.dma_start(out=g["ones_row"], in_=g["ones"].ap()[0:1, :])
            g["ones_rf"] = mktile(const, [1, 128], F32R, "ones_rf")
            nc.sync.dma_start(out=g["ones_rf"], in_=g["ones_f"].ap().bitcast(F32R))
            for nm, sh in (("b1", [128, 12]), ("bp", [128, KC]),
                           ("b3", [128, 24]), ("b4", [128, KC])):
                t = const.tile(sh, F32, tag=nm + "sb")
                nc.sync.dma_start(out=t, in_=g[nm].ap())
                g[nm + "_sb"] = t
            g["bv_sb"] = mktile(const, [1, E], BF16, "bvsb")
            nc.sync.dma_start(out=g["bv_sb"], in_=g["bv"].ap())
            g["eps_sb"] = mktile(const, [128, 1], F32, "eps_sb")
            nc.vector.memset(g["eps_sb"], EPS)
            body(nc, tc, g)
    nc.compile()
    return nc


def ln_chunk(nc, g, pools, src_tiles, dst_tiles, sl, single_bank=False,
             stats_tag="ps_sum"):
    """One 512-token LN chunk: dst = (src - mean) * rstd. Stats over the
    E=768 features (partition dim across the KC tiles) via ones-matmuls,
    broadcast to all partitions. x^2 runs on ACT (Square) to unload DVE.
    single_bank=True serializes the sum/sq accumulations through one PSUM
    bank (for use inside the attention region where banks are scarce)."""
    psA, sqp, tmp = pools
    ones128 = g["ones128"]
    ps_sum = mktile(psA, [128, 512], F32, stats_tag)
    for k in range(KC):
        nc.tensor.matmul(ps_sum[:], ones128[:], src_tiles[k][:, sl],
                         start=(k == 0), stop=(k == KC - 1))
    # mu (bf16) and mu^2 (f32) from ps_sum; var = ps_sq/E - mu^2
    mu_bf = mktile(tmp, [128, 512], BF16, "mu_bf")
    nc.scalar.activation(mu_bf, ps_sum[:], ACTF.Identity, scale=1.0 / E)
    mu2 = mktile(tmp, [128, 512], F32, "mu2")
    nc.scalar.activation(mu2, ps_sum[:], ACTF.Square, scale=1.0 / E)
    ps_sq = ps_sum if single_bank else mktile(psA, [128, 512], F32, "ps_sum2")
    for k in range(KC):
        sq = mktile(sqp, [128, 512], BF16, "sq")
        nc.vector.tensor_tensor(sq, src_tiles[k][:, sl], src_tiles[k][:, sl], ALU.mult)
        nc.tensor.matmul(ps_sq[:], ones128[:], sq[:],
                         start=(k == 0), stop=(k == KC - 1))
    varp = mktile(tmp, [128, 512], F32, "varp")
    nc.vector.scalar_tensor_tensor(varp, ps_sq[:], 1.0 / E, mu2[:],
                                   ALU.mult, ALU.subtract)
    sd = mktile(tmp, [128, 512], F32, "sd")
    nc.scalar.activation(sd, varp[:], ACTF.Sqrt, bias=g["eps_sb"][:])
    rstd = mktile(tmp, [128, 512], F32, "rstd")
    nc.vector.reciprocal_approx_fast(out=rstd, in_=sd[:])
    rstd_bf = mktile(tmp, [128, 512], BF16, "rstd_bf")
    nc.vector.tensor_copy(rstd_bf, rstd[:])
    for k in range(KC):
        d1 = mktile(tmp, [128, 512], BF16, "d1")
        nc.vector.tensor_tensor(d1, src_tiles[k][:, sl], mu_bf[:], ALU.subtract)
        nc.vector.tensor_tensor(dst_tiles[k][:, sl], d1[:], rstd_bf[:], ALU.mult)


def ln_standardize(nc, tc, g, src_tiles, dst_tiles, ntok, tag):
    # bufs=1: LN stats use only 2 PSUM banks so the downstream stage's pools
    # (emitted while later LN chunks still execute) get non-aliasing banks.
    with tc.tile_pool(name=f"psA_{tag}", bufs=1, space="PSUM") as psA, \
         tc.tile_pool(name=f"sq_{tag}", bufs=3) as sqp, \
         tc.tile_pool(name=f"lntmp_{tag}", bufs=3) as tmp:
        for t in range(ntok // 512):
            ln_chunk(nc, g, (psA, sqp, tmp), src_tiles, dst_tiles,
                     slice(t * 512, (t + 1) * 512))


def body(nc, tc, g):
    ones128, ones_row, ones_rf = g["ones128"], g["ones_row"], g["ones_rf"]

    with contextlib.ExitStack() as ctx:
        # ----- persistent qkv tiles (freed after attention) -----
        qkv_stack = ctx.enter_context(contextlib.ExitStack())
        qkv_pool = qkv_stack.enter_context(tc.tile_pool(name="qkv", bufs=1, side="right"))
        qT = [mktile(qkv_pool, [128, CHUNK], BF16, f"qT{m}") for m in range(KC)]
        kT = [mktile(qkv_pool, [128, EXT], BF16, f"kT{m}") for m in range(KC)]
        vpad = mktile(qkv_pool, [128, NT_EXT, H, D + 1], BF16, "vpad")
        nc.vector.memset(vpad[:, :, :, D:D + 1], 1.0)

        # x input (bf16, feature-major, ext tokens); freed after QKV — the
        # c_proj residual slices are re-DMA'd on demand.
        x_stack = ctx.enter_context(contextlib.ExitStack())
        xp = x_stack.enter_context(tc.tile_pool(name="xTp", bufs=1, side="right"))
        xTs = [mktile(xp, [128, EXT], BF16, f"xT{k}") for k in range(KC)]
        for c in range(3):
            for k in range(KC):
                nc.sync.dma_start(
                    out=xTs[k][:, c * 512:(c + 1) * 512],
                    in_=g["xT"].ap()[k * 128:(k + 1) * 128, c * 512:(c + 1) * 512])

        # ========== stage A: LN1 ==========
        hat_stack = ctx.enter_context(contextlib.ExitStack())
        hp = hat_stack.enter_context(tc.tile_pool(name="xhatT", bufs=1))
        xhatT = [mktile(hp, [128, EXT], BF16, f"xh{k}") for k in range(KC)]
        ln_standardize(nc, tc, g, xTs, xhatT, EXT, "ln1")

        # ========== stage B: QKV projections ==========
        # Emission ordered by xhat chunk availability so attention qb0/qb1
        # unblock after ext chunk 1: per ext chunk c emit K(c), V(tcv in c),
        # then Q(core chunk c-1) which spans xhat chunks c-1..c.
        with tc.tile_pool(name="w1p", bufs=1) as w1p, \
             tc.tile_pool(name="psQK", bufs=4, space="PSUM") as psQK, \
             tc.tile_pool(name="psV", bufs=2, space="PSUM") as psV:
            wq = [mktile(w1p, [128, 768], BF16, f"w1q{k}") for k in range(KC)]
            wk = [mktile(w1p, [128, 768], BF16, f"w1k{k}") for k in range(KC)]
            wv = [mktile(w1p, [128, 768], BF16, f"w1v{k}") for k in range(KC)]
            for k in range(KC):
                for gi, wg in ((1, wk), (2, wv), (0, wq)):
                    nc.sync.dma_start(
                        out=wg[k],
                        in_=g["w1"].ap()[k * 128:(k + 1) * 128, gi * 768:(gi + 1) * 768])

            for c in range(3):
                # K for ext chunk c
                for ml in range(6):
                    ps = mktile(psQK, [128, 512], F32, "ps_qk")
                    for k in range(KC):
                        nc.tensor.matmul(ps[:], wk[k][:, ml * 128:(ml + 1) * 128],
                                         xhatT[k][:, c * 512:(c + 1) * 512],
                                         start=(k == 0), stop=(k == KC - 1))
                    nc.scalar.activation(
                        kT[ml][:, c * 512:(c + 1) * 512], ps[:], ACTF.Identity,
                        bias=g["b1_sb"][:, 6 + ml:7 + ml])
                # V for the four 128-token chunks in ext chunk c
                for t in range(4 * c, 4 * c + 4):
                    pv = [mktile(psV, [128, 384], F32, f"ps_v{n}") for n in range(2)]
                    for k in range(KC):
                        for n in range(2):
                            nc.tensor.matmul(pv[n][:],
                                             xhatT[k][:, t * 128:(t + 1) * 128],
                                             wv[k][:, n * 384:(n + 1) * 384],
                                             start=(k == 0), stop=(k == KC - 1))
                    for n in range(2):
                        nc.tensor.matmul(pv[n][:], ones_row[:],
                                         g["bv_sb"][:, n * 384:(n + 1) * 384],
                                         start=False, stop=True, skip_group_check=True)
                        nc.vector.tensor_copy(
                            vpad[:, t, n * 6:(n + 1) * 6, 0:D],
                            pv[n][:].rearrange("p (h d) -> p h d", h=6))
                # Q for core chunk c-1 (ext cols c*512-256 : c*512+256)
                if c >= 1:
                    lo = c * 512 - 256
                    for ml in range(6):
                        ps = mktile(psQK, [128, 512], F32, "ps_qk")
                        for k in range(KC):
                            nc.tensor.matmul(ps[:], wq[k][:, ml * 128:(ml + 1) * 128],
                                             xhatT[k][:, lo:lo + 512],
                                             start=(k == 0), stop=(k == KC - 1))
                        nc.scalar.activation(
                            qT[ml][:, (c - 1) * 512:c * 512], ps[:], ACTF.Identity,
                            bias=g["b1_sb"][:, ml:ml + 1])
        hat_stack.close()   # xhatT no longer needed

        # ========== stage C: attention (qb-outer for downstream overlap) ====
        # Per (qb, pair): trimmed band scores for 256 queries x 6 key chunks
        # (kc0 only covers queries 0:128, kc5 only 128:256 -> 1280 columns),
        # exp on ACT, 0/1 band-mask multiply on DVE, PV accumulated into one
        # [65, 512] PSUM tile (both heads; row 64 = softmax denominator via
        # the ones-column in vpad), then denom reciprocal + K=1 broadcast
        # matmul + per-head scale into aT.
        at_stack = ctx.enter_context(contextlib.ExitStack())
        ap_pool = at_stack.enter_context(tc.tile_pool(name="aT", bufs=1, side="right"))
        aT = [mktile(ap_pool, [128, CHUNK], BF16, f"aT{m}") for m in range(KC)]
        x1_stack = ctx.enter_context(contextlib.ExitStack())
        x1p = x1_stack.enter_context(tc.tile_pool(name="x1T", bufs=1))
        x1T = [mktile(x1p, [128, CHUNK], BF16, f"x1{m}") for m in range(KC)]
        h2_stack = ctx.enter_context(contextlib.ExitStack())
        h2p = h2_stack.enter_context(tc.tile_pool(name="xhat2", bufs=1))
        xhat2T = [mktile(h2p, [128, CHUNK], BF16, f"x2{m}") for m in range(KC)]
        with tc.tile_pool(name="masks", bufs=1) as mp, \
             tc.tile_pool(name="wpp", bufs=1) as wpp, \
             tc.tile_pool(name="psS", bufs=3, space="PSUM") as psS, \
             tc.tile_pool(name="psO", bufs=2, space="PSUM") as psO, \
             tc.tile_pool(name="psB", bufs=1, space="PSUM") as psB, \
             tc.tile_pool(name="psD1", bufs=2, space="PSUM") as psD1, \
             tc.tile_pool(name="sq2", bufs=2) as sq2, \
             tc.tile_pool(name="tmp2", bufs=2) as tmp2, \
             tc.tile_pool(name="pP", bufs=16) as pP, \
             tc.tile_pool(name="drp", bufs=8) as drp, \
             tc.tile_pool(name="rec", bufs=8) as rp:
            m01_sb = [mktile(mp, [128, 384], BF16, f"m01_{qb}") for qb in range(4)]
            m45_sb = [mktile(mp, [128, 384], BF16, f"m45_{qb}") for qb in range(4)]
            for qb in range(4):
                nc.gpsimd.dma_start(out=m01_sb[qb], in_=g["m01"].ap()[qb])
                nc.gpsimd.dma_start(out=m45_sb[qb], in_=g["m45"].ap()[qb])
            wps = [mktile(wpp, [128, E], BF16, f"wp{k}") for k in range(KC)]
            for k in range(KC):
                nc.gpsimd.dma_start(out=wps[k], in_=g["wp"].ap()[k * 128:(k + 1) * 128, :])

            # (ps columns, p columns, po column range, q column range) per kc
            kc_map = [
                (0, 128, (0, 128), (0, 128)),       # kc0: queries 0:128
                (128, 384, (0, 256), (0, 256)),     # kc1
                (0, 256, (0, 256), (0, 256)),       # kc2
                (256, 512, (0, 256), (0, 256)),     # kc3
                (0, 256, (0, 256), (0, 256)),       # kc4
                (256, 384, (128, 256), (128, 256)), # kc5: queries 128:256
            ]
            def pv_mms(qb, pair, pT, po, h, kcp):
                for j in range(2):
                    kc = 2 * kcp + j
                    tcv = 2 * qb + kc
                    c0, c1, (o0, o1), _ = kc_map[kc]
                    nc.tensor.matmul(
                        po[:, h * 256 + o0:h * 256 + o1],
                        vpad[:, tcv, 2 * pair + h, :],
                        pT[(kcp, h)][:, c0:c1],
                        start=(h == 0 and kc == 0), stop=(h == 1 and kc == 5),
                        skip_group_check=True)

            def emit_denom_scale(qb, pair, po):
                qbase = qb * 256
                # denominator row -> SBUF bf16, broadcast to all partitions
                # via K=1 matmul, then reciprocal of the broadcast tile
                drow = mktile(drp, [1, 512], BF16, "drow")
                nc.vector.tensor_copy(drow, po[64:65, :])
                bb_ps = mktile(psB, [128, 512], F32, "bb_ps")
                nc.tensor.matmul(bb_ps[:], ones_row[:], drow[:],
                                 start=True, stop=True)
                rbb = mktile(rp, [128, 512], F32, "rbb")
                nc.vector.reciprocal_approx_fast(out=rbb, in_=bb_ps[:])
                for h in range(2):
                    nc.vector.tensor_tensor(
                        aT[pair][h * 64:(h + 1) * 64, qbase:qbase + 256],
                        po[0:64, h * 256:(h + 1) * 256],
                        rbb[h * 64:(h + 1) * 64, h * 256:(h + 1) * 256], ALU.mult)

            def emit_cproj(qb):
                qbase = qb * 256
                for m in range(KC):
                    ps = mktile(psD1, [128, 256], F32, "ps_d1")
                    for k in range(KC):
                        nc.tensor.matmul(ps[:], wps[k][:, m * 128:(m + 1) * 128],
                                         aT[k][:, qbase:qbase + 256],
                                         start=(k == 0), stop=(k == KC - 1))
                    nc.vector.scalar_tensor_tensor(
                        x1T[m][:, qbase:qbase + 256], ps[:], g["bp_sb"][:, m:m + 1],
                        xTs[m][:, WIN + qbase:WIN + qbase + 256], ALU.add, ALU.add)

            # software pipeline, tile-granular: each (h, kcp) step emits the
            # 2 score MMs of iteration i, then the matching 2 PV MMs of
            # iteration i-1, so the PE stream always has ready work while
            # ACT's exp drains the score tiles.
            iters = [(qb, pair) for qb in range(4) for pair in range(KC)]
            prev = None          # (qb, pair, pT, po)
            for idx, it in enumerate(iters):
                qb, pair = it
                qbase = qb * 256
                pT = {}
                po = mktile(psO, [65, 512], F32, "po")
                for h in range(2):
                    for kcp in range(3):
                        ncol = 512 if kcp == 1 else 384
                        ps_s = mktile(psS, [128, 512], F32, "ps_s")
                        for j in range(2):
                            kc = 2 * kcp + j
                            tcv = 2 * qb + kc
                            c0, c1, _, (q0, q1) = kc_map[kc]
                            nc.tensor.matmul(
                                ps_s[:, c0:c1],
                                kT[pair][h * 64:(h + 1) * 64, tcv * 128:(tcv + 1) * 128],
                                qT[pair][h * 64:(h + 1) * 64, qbase + q0:qbase + q1],
                                start=True, stop=True, tile_position=(h * 64, 0),
                                skip_group_check=True)
                        p = mktile(pP, [128, 512], BF16, "pT")
                        nc.scalar.activation(p[:, 0:ncol], ps_s[:, 0:ncol], ACTF.Exp)
                        if kcp == 0:
                            nc.vector.tensor_tensor(p[:, 0:384], p[:, 0:384],
                                                    m01_sb[qb][:], ALU.mult)
                        elif kcp == 2:
                            nc.vector.tensor_tensor(p[:, 0:384], p[:, 0:384],
                                                    m45_sb[qb][:], ALU.mult)
                        pT[(kcp, h)] = p
                        if prev is not None:
                            pv_mms(prev[0], prev[1], prev[2], prev[3], h, kcp)
                if prev is not None:
                    emit_denom_scale(prev[0], prev[1], prev[3])
                    if prev[1] == KC - 1:
                        emit_cproj(prev[0])
                prev = (qb, pair, pT, po)
            for h in range(2):
                for kcp in range(3):
                    pv_mms(prev[0], prev[1], prev[2], prev[3], h, kcp)
            emit_denom_scale(prev[0], prev[1], prev[3])
            emit_cproj(3)
        at_stack.close()    # aT freed
        x_stack.close()     # xTs freed
        qkv_stack.close()   # qT/kT/vpad freed

        # ========== stage D: LN2 + FFN ==========
        ln_standardize(nc, tc, g, x1T, xhat2T, CHUNK, "ln2")
        with tc.tile_pool(name="w3p", bufs=1) as w3p, \
             tc.tile_pool(name="w4p", bufs=1) as w4p, \
             tc.tile_pool(name="fTp", bufs=1) as fp, \
             tc.tile_pool(name="psF1", bufs=4, space="PSUM") as psF1, \
             tc.tile_pool(name="psF2", bufs=4, space="PSUM") as psF2, \
             tc.tile_pool(name="onat", bufs=3) as onp:
            fT = [mktile(fp, [128, 24, 512], BF16, "fT0"),
                  mktile(fp, [128, 24, 512], BF16, "fT1")]
            w3g = [[mktile(w3p, [128, 768], BF16, f"w3g{gi}_{k}") for k in range(KC)]
                   for gi in range(4)]
            w4g = [[mktile(w4p, [128, E], BF16, f"w4g{gi}_{k}") for k in range(KC)]
                   for gi in range(4)]
            for gi in range(4):
                for k in range(KC):
                    nc.gpsimd.dma_start(
                        out=w3g[gi][k],
                        in_=g["w3"].ap()[k * 128:(k + 1) * 128, gi * 768:(gi + 1) * 768])
                    nc.gpsimd.dma_start(
                        out=w4g[gi][k],
                        in_=g["w4"].ap()[(gi * 6 + k) * 128:(gi * 6 + k + 1) * 128, :])

            def ffn1_chain(m, t, pool, tag):
                sl = slice(t * 512, (t + 1) * 512)
                ps = mktile(pool, [128, 512], F32, tag)
                for k in range(KC):
                    nc.tensor.matmul(ps[:], w3g[m // 6][k][:, (m % 6) * 128:(m % 6 + 1) * 128],
                                     xhat2T[k][:, sl],
                                     start=(k == 0), stop=(k == KC - 1))
                nc.scalar.activation(fT[t][:, m, :], ps[:], ACTF.Gelu,
                                     bias=g["b3_sb"][:, m:m + 1])

            def ffn2_chain(m, t):
                sl = slice(t * 512, (t + 1) * 512)
                ps = mktile(psF2, [128, 512], F32, "ps_f2")
                for ch in range(24):
                    nc.tensor.matmul(ps[:], w4g[ch // 6][ch % 6][:, m * 128:(m + 1) * 128],
                                     fT[t][:, ch, :],
                                     start=(ch == 0), stop=(ch == 23))
                onat = mktile(onp, [128, 512], BF16, "onat")
                nc.vector.scalar_tensor_tensor(
                    onat, ps[:], g["b4_sb"][:, m:m + 1], x1T[m][:, sl],
                    ALU.add, ALU.add)
                nc.sync.dma_start(
                    out=g["out"].ap()[m * 128:(m + 1) * 128, t * 512:(t + 1) * 512],
                    in_=onat[:])

            for m in range(24):
                ffn1_chain(m, 0, psF1, "ps_f1")
            for m in range(KC):
                ffn2_chain(m, 0)
            for m in range(24):
                ffn1_chain(m, 1, psF1, "ps_f1")
            for m in range(KC):
                ffn2_chain(m, 1)


# ---------------------------------------------------------------------------
# host side
# ---------------------------------------------------------------------------

def _build_masks(s_idx):
    """Trimmed band masks, bf16. m01: [4(qb), 128, 384] with cols 0:128 = kc0
    (queries 0:128 of the block) and cols 128:384 = kc1 (queries 0:256).
    m45: cols 0:256 = kc4 (queries 0:256), cols 256:384 = kc5 (queries
    128:256). 1.0 keep, 0.0 drop."""
    p = np.arange(128)[:, None]          # key index within 128-chunk
    m01 = np.zeros((4, 128, 384), np.float32)
    m45 = np.zeros((4, 128, 384), np.float32)
    for qb in range(4):
        c_g = s_idx * 4 + qb

        def valid(kc, x):
            y = kc * 128 + p                      # window-local key pos (0..767)
            jg = c_g * 256 - 256 + y              # global key index
            ok = (y >= x) & (y <= x + 2 * WIN) & (jg >= 0) & (jg < S)
            return ok.astype(np.float32)

        m01[qb, :, 0:128] = valid(0, np.arange(128)[None, :])
        m01[qb, :, 128:384] = valid(1, np.arange(256)[None, :])
        m45[qb, :, 0:256] = valid(4, np.arange(256)[None, :])
        m45[qb, :, 256:384] = valid(5, np.arange(128, 256)[None, :])
    return m01, m45


_built = {}


def _get_nc():
    if "nc" not in _built:
        _built["nc"] = build()
    return _built["nc"]


def _bf16(x):
    import ml_dtypes
    return np.ascontiguousarray(np.asarray(x, np.float32).astype(ml_dtypes.bfloat16))


def make_in_maps(x, ln1_g, ln1_b, c_attn_w, c_attn_b, c_proj_w, c_proj_b,
                 ln2_g, ln2_b, fc_w, fc_b, proj2_w, proj2_b, w):
    assert int(w) == WIN
    f64 = np.float64
    w1 = (np.asarray(ln1_g, f64)[:, None] * np.asarray(c_attn_w, f64))
    bqkv = (np.asarray(ln1_b, f64) @ np.asarray(c_attn_w, f64)
            + np.asarray(c_attn_b, f64)).copy()
    w1[:, :E] *= 1.0 / np.sqrt(D)
    bqkv[:E] *= 1.0 / np.sqrt(D)
    w3 = (np.asarray(ln2_g, f64)[:, None] * np.asarray(fc_w, f64))
    b3 = np.asarray(ln2_b, f64) @ np.asarray(fc_w, f64) + np.asarray(fc_b, f64)

    common = {
        "w1": _bf16(w1),
        "b1": np.ascontiguousarray(
            np.asarray(bqkv[:2 * E], np.float32).reshape(12, 128).T),
        "bv": _bf16(bqkv[None, 2 * E:]),
        "wp": _bf16(c_proj_w),
        "bp": np.ascontiguousarray(
            np.asarray(c_proj_b, np.float32).reshape(KC, 128).T),
        "w3": _bf16(w3),
        "b3": np.ascontiguousarray(np.asarray(b3, np.float32).reshape(24, 128).T),
        "w4": _bf16(proj2_w),
        "b4": np.ascontiguousarray(
            np.asarray(proj2_b, np.float32).reshape(KC, 128).T),
        "ones": _bf16(np.ones((128, 128), np.float32)),
        "ones_f": np.ones((1, 128), np.float32),
    }
    masks = [_build_masks(s) for s in range(NSEQ)]
    x = np.asarray(x, np.float32)
    in_maps = []
    for ci in range(8):
        b, s = divmod(ci, NSEQ)
        xt = np.zeros((E, EXT), np.float32)
        lo = s * CHUNK - WIN
        hi = s * CHUNK + CHUNK + WIN
        slo, shi = max(lo, 0), min(hi, S)
        xt[:, slo - lo:shi - lo] = x[b, slo:shi, :].T
        m01, m45 = masks[s]
        in_maps.append(dict(common, xT=_bf16(xt), m01=_bf16(m01), m45=_bf16(m45)))
    return in_maps


def assemble(results):
    out = np.empty((B, S, E), np.float32)
    for ci in range(8):
        b, s = divmod(ci, NSEQ)
        out[b, s * CHUNK:(s + 1) * CHUNK, :] = np.asarray(
            results[ci]["out"], np.float32).T
    return out


def kernel(**inputs):
    in_maps = make_in_maps(**inputs)
    nc = _get_nc()
    res = run_bass_kernel_spmd(nc, in_maps, core_ids=list(range(8)))
    return assemble(res.results)


# revision 67
# speedup vs baseline: 1.0245x; 1.0245x over previous
"""Trainium2 Bass kernel for a GPT-style block with sliding-window attention.

Sharding: 8 cores = batch(2) x sequence-quarters(4). Each core processes its
1024 tokens end-to-end (LN1 -> QKV -> windowed attention -> proj -> residual ->
LN2 -> FFN(gelu) -> residual), with a 256-token halo recomputed for K/V.
No collectives. Activations are feature-major (features on partitions, tokens
on the free dim) so every matmul chains directly; output stays feature-major
and is transposed on the host.

v2 (vs baseline): bf16 weights+activations (f32 PSUM/stats), attention band
trimmed to the 1280 valid score columns per 256-query block, softmax
denominator normalized via reciprocal_approx_fast + a K=1 broadcast matmul
(no DRAM bounce), qb-outer attention ordering so c_proj/LN2/FFN overlap, FFN2
accumulated 24-deep in PSUM (no DVE accumulation), and no output transpose.
"""
import contextlib
import numpy as np

import concourse.bass as bass
import concourse.mybir as mybir
import concourse.tile as tile
from concourse import bacc
from concourse.bass_utils import run_bass_kernel_spmd

F32R = mybir.dt.float32r
F32 = mybir.dt.float32
BF16 = mybir.dt.bfloat16
ALU = mybir.AluOpType
ACTF = mybir.ActivationFunctionType

B, S, E, H, D, WIN = 2, 4096, 768, 12, 64, 256
NSEQ = 4                      # sequence shards per batch
CHUNK = S // NSEQ             # 1024 core tokens per core
EXT = CHUNK + 2 * WIN         # 1536 extended tokens (k/v halo)
KC = E // 128                 # 6 chunks of the embedding dim
NT_EXT = EXT // 128           # 12
EPS = 1e-5


def mktile(pool, shape, dtype, tag):
    return pool.tile(shape, dtype, tag=tag, name=tag)


def build():
    nc = bacc.Bacc("TRN2", target_bir_lowering=False, debug=False, num_devices=8)

    g = {}
    g["xT"] = nc.dram_tensor("xT", [E, EXT], BF16, kind="ExternalInput")
    g["w1"] = nc.dram_tensor("w1", [E, 3 * E], BF16, kind="ExternalInput")
    g["b1"] = nc.dram_tensor("b1", [128, 12], F32, kind="ExternalInput")
    g["bv"] = nc.dram_tensor("bv", [1, E], BF16, kind="ExternalInput")
    g["wp"] = nc.dram_tensor("wp", [E, E], BF16, kind="ExternalInput")
    g["bp"] = nc.dram_tensor("bp", [128, KC], F32, kind="ExternalInput")
    g["w3"] = nc.dram_tensor("w3", [E, 4 * E], BF16, kind="ExternalInput")
    g["b3"] = nc.dram_tensor("b3", [128, 24], F32, kind="ExternalInput")
    g["w4"] = nc.dram_tensor("w4", [4 * E, E], BF16, kind="ExternalInput")
    g["b4"] = nc.dram_tensor("b4", [128, KC], F32, kind="ExternalInput")
    g["m01"] = nc.dram_tensor("m01", [4, 128, 384], BF16, kind="ExternalInput")
    g["m45"] = nc.dram_tensor("m45", [4, 128, 384], BF16, kind="ExternalInput")
    g["ones"] = nc.dram_tensor("ones", [128, 128], BF16, kind="ExternalInput")
    g["ones_f"] = nc.dram_tensor("ones_f", [1, 128], F32, kind="ExternalInput")
    g["out"] = nc.dram_tensor("out", [E, CHUNK], BF16, kind="ExternalOutput")

    with tile.TileContext(nc) as tc:
        with tc.tile_pool(name="const", bufs=1) as const:
            g["ones128"] = mktile(const, [128, 128], BF16, "ones128")
            nc.gpsimd.dma_start(out=g["ones128"], in_=g["ones"].ap())
            g["ones_row"] = mktile(const, [1, 128], BF16, "ones_row")
            nc.gpsimd.dma_start(out=g["ones_row"], in_=g["ones"].ap()[0:1, :])
            g["ones_rf"] = mktile(const, [1, 128], F32R, "ones_rf")
            nc.gpsimd.dma_start(out=g["ones_rf"], in_=g["ones_f"].ap().bitcast(F32R))
            for nm, sh in (("b1", [128, 12]), ("bp", [128, KC]),
                           ("b3", [128, 24]), ("b4", [128, KC])):
                t = const.tile(sh, F32, tag=nm + "sb")
                nc.gpsimd.dma_start(out=t, in_=g[nm].ap())
                g[nm + "_sb"] = t
            g["bv_sb"] = mktile(const, [1, E], BF16, "bvsb")
            nc.gpsimd.dma_start(out=g["bv_sb"], in_=g["bv"].ap())
            g["eps_sb"] = mktile(const, [128, 1], F32, "eps_sb")
            nc.vector.memset(g["eps_sb"], EPS)
            body(nc, tc, g)
    nc.compile()
    return nc


def ln_chunk(nc, g, pools, src_tiles, dst_tiles, sl, single_bank=False,
             stats_tag="ps_sum"):
    """One 512-token LN chunk: dst = (src - mean) * rstd. Stats over the
    E=768 features (partition dim across the KC tiles) via ones-matmuls,
    broadcast to all partitions. x^2 runs on ACT (Square) to unload DVE.
    single_bank=True serializes the sum/sq accumulations through one PSUM
    bank (for use inside the attention region where banks are scarce)."""
    psA, sqp, tmp = pools
    ones128 = g["ones128"]
    ps_sum = mktile(psA, [128, 512], F32, stats_tag)
    for k in range(KC):
        nc.tensor.matmul(ps_sum[:], ones128[:], src_tiles[k][:, sl],
                         start=(k == 0), stop=(k == KC - 1))
    # mu (bf16) and mu^2 (f32) from ps_sum; var = ps_sq/E - mu^2
    mu_bf = mktile(tmp, [128, 512], BF16, "mu_bf")
    nc.scalar.activation(mu_bf, ps_sum[:], ACTF.Identity, scale=1.0 / E)
    mu2 = mktile(tmp, [128, 512], F32, "mu2")
    nc.scalar.activation(mu2, ps_sum[:], ACTF.Square, scale=1.0 / E)
    ps_sq = ps_sum if single_bank else mktile(psA, [128, 512], F32, "ps_sum2")
    for k in range(KC):
        sq = mktile(sqp, [128, 512], BF16, "sq")
        nc.vector.tensor_tensor(sq, src_tiles[k][:, sl], src_tiles[k][:, sl], ALU.mult)
        nc.tensor.matmul(ps_sq[:], ones128[:], sq[:],
                         start=(k == 0), stop=(k == KC - 1))
    varp = mktile(tmp, [128, 512], F32, "varp")
    nc.vector.scalar_tensor_tensor(varp, ps_sq[:], 1.0 / E, mu2[:],
                                   ALU.mult, ALU.subtract)
    sd = mktile(tmp, [128, 512], F32, "sd")
    nc.scalar.activation(sd, varp[:], ACTF.Sqrt, bias=g["eps_sb"][:])
    rstd = mktile(tmp, [128, 512], F32, "rstd")
    nc.vector.reciprocal_approx_fast(out=rstd, in_=sd[:])
    rstd_bf = mktile(tmp, [128, 512], BF16, "rstd_bf")
    nc.vector.tensor_copy(rstd_bf, rstd[:])
    for k in range(KC):
        d1 = mktile(tmp, [128, 512], BF16, "d1")
        nc.vector.tensor_tensor(d1, src_tiles[k][:, sl], mu_bf[:], ALU.subtract)
        nc.vector.tensor_tensor(dst_tiles[k][:, sl], d1[:], rstd_bf[:], ALU.mult)


def ln_standardize(nc, tc, g, src_tiles, dst_tiles, ntok, tag):
    # bufs=1: LN stats use only 2 PSUM banks so the downstream stage's pools
    # (emitted while later LN chunks still execute) get non-aliasing banks.
    with tc.tile_pool(name=f"psA_{tag}", bufs=1, space="PSUM") as psA, \
         tc.tile_pool(name=f"sq_{tag}", bufs=3) as sqp, \
         tc.tile_pool(name=f"lntmp_{tag}", bufs=3) as tmp:
        for t in range(ntok // 512):
            ln_chunk(nc, g, (psA, sqp, tmp), src_tiles, dst_tiles,
                     slice(t * 512, (t + 1) * 512))


def body(nc, tc, g):
    ones128, ones_row, ones_rf = g["ones128"], g["ones_row"], g["ones_rf"]

    with contextlib.ExitStack() as ctx:
        # ----- persistent qkv tiles (freed after attention) -----
        qkv_stack = ctx.enter_context(contextlib.ExitStack())
        qkv_pool = qkv_stack.enter_context(tc.tile_pool(name="qkv", bufs=1, side="right"))
        qT = [mktile(qkv_pool, [128, CHUNK], BF16, f"qT{m}") for m in range(KC)]
        kT = [mktile(qkv_pool, [128, EXT], BF16, f"kT{m}") for m in range(KC)]
        vpad = mktile(qkv_pool, [128, NT_EXT, H, D + 1], BF16, "vpad")
        nc.vector.memset(vpad[:, :, :, D:D + 1], 1.0)

        # x input (bf16, feature-major, ext tokens); freed after QKV — the
        # c_proj residual slices are re-DMA'd on demand.
        x_stack = ctx.enter_context(contextlib.ExitStack())
        xp = x_stack.enter_context(tc.tile_pool(name="xTp", bufs=1, side="right"))
        xTs = [mktile(xp, [128, EXT], BF16, f"xT{k}") for k in range(KC)]
        for c in range(3):
            for k in range(KC):
                nc.sync.dma_start(
                    out=xTs[k][:, c * 512:(c + 1) * 512],
                    in_=g["xT"].ap()[k * 128:(k + 1) * 128, c * 512:(c + 1) * 512])

        # ========== stage A: LN1 ==========
        hat_stack = ctx.enter_context(contextlib.ExitStack())
        hp = hat_stack.enter_context(tc.tile_pool(name="xhatT", bufs=1))
        xhatT = [mktile(hp, [128, EXT], BF16, f"xh{k}") for k in range(KC)]
        ln_standardize(nc, tc, g, xTs, xhatT, EXT, "ln1")

        # ========== stage B: QKV projections ==========
        # Emission ordered by xhat chunk availability so attention qb0/qb1
        # unblock after ext chunk 1: per ext chunk c emit K(c), V(tcv in c),
        # then Q(core chunk c-1) which spans xhat chunks c-1..c.
        with tc.tile_pool(name="w1p", bufs=1) as w1p, \
             tc.tile_pool(name="psQK", bufs=4, space="PSUM") as psQK, \
             tc.tile_pool(name="psV", bufs=2, space="PSUM") as psV:
            wq = [mktile(w1p, [128, 768], BF16, f"w1q{k}") for k in range(KC)]
            wk = [mktile(w1p, [128, 768], BF16, f"w1k{k}") for k in range(KC)]
            wv = [mktile(w1p, [128, 768], BF16, f"w1v{k}") for k in range(KC)]
            for k in range(KC):
                for gi, wg in ((1, wk), (2, wv), (0, wq)):
                    nc.sync.dma_start(
                        out=wg[k],
                        in_=g["w1"].ap()[k * 128:(k + 1) * 128, gi * 768:(gi + 1) * 768])

            for c in range(3):
                # K for ext chunk c
                for ml in range(6):
                    ps = mktile(psQK, [128, 512], F32, "ps_qk")
                    for k in range(KC):
                        nc.tensor.matmul(ps[:], wk[k][:, ml * 128:(ml + 1) * 128],
                                         xhatT[k][:, c * 512:(c + 1) * 512],
                                         start=(k == 0), stop=(k == KC - 1))
                    nc.scalar.activation(
                        kT[ml][:, c * 512:(c + 1) * 512], ps[:], ACTF.Identity,
                        bias=g["b1_sb"][:, 6 + ml:7 + ml])
                # V for the four 128-token chunks in ext chunk c
                for t in range(4 * c, 4 * c + 4):
                    pv = [mktile(psV, [128, 384], F32, f"ps_v{n}") for n in range(2)]
                    for k in range(KC):
                        for n in range(2):
                            nc.tensor.matmul(pv[n][:],
                                             xhatT[k][:, t * 128:(t + 1) * 128],
                                             wv[k][:, n * 384:(n + 1) * 384],
                                             start=(k == 0), stop=(k == KC - 1))
                    for n in range(2):
                        nc.tensor.matmul(pv[n][:], ones_row[:],
                                         g["bv_sb"][:, n * 384:(n + 1) * 384],
                                         start=False, stop=True, skip_group_check=True)
                        nc.vector.tensor_copy(
                            vpad[:, t, n * 6:(n + 1) * 6, 0:D],
                            pv[n][:].rearrange("p (h d) -> p h d", h=6))
                # Q for core chunk c-1 (ext cols c*512-256 : c*512+256)
                if c >= 1:
                    lo = c * 512 - 256
                    for ml in range(6):
                        ps = mktile(psQK, [128, 512], F32, "ps_qk")
                        for k in range(KC):
                            nc.tensor.matmul(ps[:], wq[k][:, ml * 128:(ml + 1) * 128],
                                             xhatT[k][:, lo:lo + 512],
                                             start=(k == 0), stop=(k == KC - 1))
                        nc.scalar.activation(
                            qT[ml][:, (c - 1) * 512:c * 512], ps[:], ACTF.Identity,
                            bias=g["b1_sb"][:, ml:ml + 1])
        hat_stack.close()   # xhatT no longer needed

        # ========== stage C: attention (qb-outer for downstream overlap) ====
        # Per (qb, pair): trimmed band scores for 256 queries x 6 key chunks
        # (kc0 only covers queries 0:128, kc5 only 128:256 -> 1280 columns),
        # exp on ACT, 0/1 band-mask multiply on DVE, PV accumulated into one
        # [65, 512] PSUM tile (both heads; row 64 = softmax denominator via
        # the ones-column in vpad), then denom reciprocal + K=1 broadcast
        # matmul + per-head scale into aT.
        at_stack = ctx.enter_context(contextlib.ExitStack())
        ap_pool = at_stack.enter_context(tc.tile_pool(name="aT", bufs=1, side="right"))
        aT = [mktile(ap_pool, [128, CHUNK], BF16, f"aT{m}") for m in range(KC)]
        x1_stack = ctx.enter_context(contextlib.ExitStack())
        x1p = x1_stack.enter_context(tc.tile_pool(name="x1T", bufs=1))
        x1T = [mktile(x1p, [128, CHUNK], BF16, f"x1{m}") for m in range(KC)]
        h2_stack = ctx.enter_context(contextlib.ExitStack())
        h2p = h2_stack.enter_context(tc.tile_pool(name="xhat2", bufs=1))
        xhat2T = [mktile(h2p, [128, CHUNK], BF16, f"x2{m}") for m in range(KC)]
        with tc.tile_pool(name="masks", bufs=1) as mp, \
             tc.tile_pool(name="wpp", bufs=1) as wpp, \
             tc.tile_pool(name="psS", bufs=3, space="PSUM") as psS, \
             tc.tile_pool(name="psO", bufs=2, space="PSUM") as psO, \
             tc.tile_pool(name="psB", bufs=1, space="PSUM") as psB, \
             tc.tile_pool(name="psD1", bufs=2, space="PSUM") as psD1, \
             tc.tile_pool(name="sq2", bufs=2) as sq2, \
             tc.tile_pool(name="tmp2", bufs=2) as tmp2, \
             tc.tile_pool(name="pP", bufs=16) as pP, \
             tc.tile_pool(name="drp", bufs=8) as drp, \
             tc.tile_pool(name="rec", bufs=8) as rp:
            m01_sb = [mktile(mp, [128, 384], BF16, f"m01_{qb}") for qb in range(4)]
            m45_sb = [mktile(mp, [128, 384], BF16, f"m45_{qb}") for qb in range(4)]
            for qb in range(4):
                nc.gpsimd.dma_start(out=m01_sb[qb], in_=g["m01"].ap()[qb])
                nc.gpsimd.dma_start(out=m45_sb[qb], in_=g["m45"].ap()[qb])
            wps = [mktile(wpp, [128, E], BF16, f"wp{k}") for k in range(KC)]
            for k in range(KC):
                nc.gpsimd.dma_start(out=wps[k], in_=g["wp"].ap()[k * 128:(k + 1) * 128, :])

            # (ps columns, p columns, po column range, q column range) per kc
            kc_map = [
                (0, 128, (0, 128), (0, 128)),       # kc0: queries 0:128
                (128, 384, (0, 256), (0, 256)),     # kc1
                (0, 256, (0, 256), (0, 256)),       # kc2
                (256, 512, (0, 256), (0, 256)),     # kc3
                (0, 256, (0, 256), (0, 256)),       # kc4
                (256, 384, (128, 256), (128, 256)), # kc5: queries 128:256
            ]
            def pv_mms(qb, pair, pT, po, h, kcp):
                for j in range(2):
                    kc = 2 * kcp + j
                    tcv = 2 * qb + kc
                    c0, c1, (o0, o1), _ = kc_map[kc]
                    nc.tensor.matmul(
                        po[:, h * 256 + o0:h * 256 + o1],
                        vpad[:, tcv, 2 * pair + h, :],
                        pT[(kcp, h)][:, c0:c1],
                        start=(h == 0 and kc == 0), stop=(h == 1 and kc == 5),
                        skip_group_check=True)

            def emit_denom_scale(qb, pair, po):
                qbase = qb * 256
                # denominator row -> SBUF bf16, broadcast to all partitions
                # via K=1 matmul, then reciprocal of the broadcast tile
                drow = mktile(drp, [1, 512], BF16, "drow")
                nc.vector.tensor_copy(drow, po[64:65, :])
                bb_ps = mktile(psB, [128, 512], F32, "bb_ps")
                nc.tensor.matmul(bb_ps[:], ones_row[:], drow[:],
                                 start=True, stop=True)
                rbb = mktile(rp, [128, 512], F32, "rbb")
                nc.vector.reciprocal_approx_fast(out=rbb, in_=bb_ps[:])
                for h in range(2):
                    nc.vector.tensor_tensor(
                        aT[pair][h * 64:(h + 1) * 64, qbase:qbase + 256],
                        po[0:64, h * 256:(h + 1) * 256],
                        rbb[h * 64:(h + 1) * 64, h * 256:(h + 1) * 256], ALU.mult)

            def emit_cproj(qb):
                qbase = qb * 256
                for m in range(KC):
                    ps = mktile(psD1, [128, 256], F32, "ps_d1")
                    for k in range(KC):
                        nc.tensor.matmul(ps[:], wps[k][:, m * 128:(m + 1) * 128],
                                         aT[k][:, qbase:qbase + 256],
                                         start=(k == 0), stop=(k == KC - 1))
                    nc.vector.scalar_tensor_tensor(
                        x1T[m][:, qbase:qbase + 256], ps[:], g["bp_sb"][:, m:m + 1],
                        xTs[m][:, WIN + qbase:WIN + qbase + 256], ALU.add, ALU.add)

            # software pipeline, tile-granular: each (h, kcp) step emits the
            # 2 score MMs of iteration i, then the matching 2 PV MMs of
            # iteration i-1, so the PE stream always has ready work while
            # ACT's exp drains the score tiles.
            iters = [(qb, pair) for qb in range(4) for pair in range(KC)]
            prev = None          # (qb, pair, pT, po)
            for idx, it in enumerate(iters):
                qb, pair = it
                qbase = qb * 256
                pT = {}
                po = mktile(psO, [65, 512], F32, "po")
                for h in range(2):
                    for kcp in range(3):
                        ncol = 512 if kcp == 1 else 384
                        ps_s = mktile(psS, [128, 512], F32, "ps_s")
                        for j in range(2):
                            kc = 2 * kcp + j
                            tcv = 2 * qb + kc
                            c0, c1, _, (q0, q1) = kc_map[kc]
                            nc.tensor.matmul(
                                ps_s[:, c0:c1],
                                kT[pair][h * 64:(h + 1) * 64, tcv * 128:(tcv + 1) * 128],
                                qT[pair][h * 64:(h + 1) * 64, qbase + q0:qbase + q1],
                                start=True, stop=True, tile_position=(h * 64, 0),
                                skip_group_check=True)
                        p = mktile(pP, [128, 512], BF16, "pT")
                        nc.scalar.activation(p[:, 0:ncol], ps_s[:, 0:ncol], ACTF.Exp)
                        if kcp == 0:
                            nc.vector.tensor_tensor(p[:, 0:384], p[:, 0:384],
                                                    m01_sb[qb][:], ALU.mult)
                        elif kcp == 2:
                            nc.vector.tensor_tensor(p[:, 0:384], p[:, 0:384],
                                                    m45_sb[qb][:], ALU.mult)
                        pT[(kcp, h)] = p
                        if prev is not None:
                            pv_mms(prev[0], prev[1], prev[2], prev[3], h, kcp)
                if prev is not None:
                    emit_denom_scale(prev[0], prev[1], prev[3])
                    if prev[1] == KC - 1:
                        emit_cproj(prev[0])
                prev = (qb, pair, pT, po)
            for h in range(2):
                for kcp in range(3):
                    pv_mms(prev[0], prev[1], prev[2], prev[3], h, kcp)
            emit_denom_scale(prev[0], prev[1], prev[3])
            emit_cproj(3)
        at_stack.close()    # aT freed
        x_stack.close()     # xTs freed
        qkv_stack.close()   # qT/kT/vpad freed

        # ========== stage D: LN2 + FFN ==========
        ln_standardize(nc, tc, g, x1T, xhat2T, CHUNK, "ln2")
        with tc.tile_pool(name="w3p", bufs=1) as w3p, \
             tc.tile_pool(name="w4p", bufs=1) as w4p, \
             tc.tile_pool(name="fTp", bufs=1) as fp, \
             tc.tile_pool(name="psF1", bufs=4, space="PSUM") as psF1, \
             tc.tile_pool(name="psF2", bufs=4, space="PSUM") as psF2, \
             tc.tile_pool(name="onat", bufs=3) as onp:
            fT = [mktile(fp, [128, 24, 512], BF16, "fT0"),
                  mktile(fp, [128, 24, 512], BF16, "fT1")]
            w3g = [[mktile(w3p, [128, 768], BF16, f"w3g{gi}_{k}") for k in range(KC)]
                   for gi in range(4)]
            w4g = [[mktile(w4p, [128, E], BF16, f"w4g{gi}_{k}") for k in range(KC)]
                   for gi in range(4)]
            for gi in range(4):
                for k in range(KC):
                    nc.gpsimd.dma_start(
                        out=w3g[gi][k],
                        in_=g["w3"].ap()[k * 128:(k + 1) * 128, gi * 768:(gi + 1) * 768])
                    nc.gpsimd.dma_start(
                        out=w4g[gi][k],
                        in_=g["w4"].ap()[(gi * 6 + k) * 128:(gi * 6 + k + 1) * 128, :])

            def ffn1_chain(m, t, pool, tag):
                sl = slice(t * 512, (t + 1) * 512)
                ps = mktile(pool, [128, 512], F32, tag)
                for k in range(KC):
                    nc.tensor.matmul(ps[:], w3g[m // 6][k][:, (m % 6) * 128:(m % 6 + 1) * 128],
                                     xhat2T[k][:, sl],
                                     start=(k == 0), stop=(k == KC - 1))
                nc.scalar.activation(fT[t][:, m, :], ps[:], ACTF.Gelu,
                                     bias=g["b3_sb"][:, m:m + 1])

            def ffn2_chain(m, t):
                sl = slice(t * 512, (t + 1) * 512)
                ps = mktile(psF2, [128, 512], F32, "ps_f2")
                for ch in range(24):
                    nc.tensor.matmul(ps[:], w4g[ch // 6][ch % 6][:, m * 128:(m + 1) * 128],
                                     fT[t][:, ch, :],
                                     start=(ch == 0), stop=(ch == 23))
                onat = mktile(onp, [128, 512], BF16, "onat")
                nc.vector.scalar_tensor_tensor(
                    onat, ps[:], g["b4_sb"][:, m:m + 1], x1T[m][:, sl],
                    ALU.add, ALU.add)
                nc.sync.dma_start(
                    out=g["out"].ap()[m * 128:(m + 1) * 128, t * 512:(t + 1) * 512],
                    in_=onat[:])

            for m in range(24):
                ffn1_chain(m, 0, psF1, "ps_f1")
            for m in range(KC):
                ffn2_chain(m, 0)
            for m in range(24):
                ffn1_chain(m, 1, psF1, "ps_f1")
            for m in range(KC):
                ffn2_chain(m, 1)


# ---------------------------------------------------------------------------
# host side
# ---------------------------------------------------------------------------

def _build_masks(s_idx):
    """Trimmed band masks, bf16. m01: [4(qb), 128, 384] with cols 0:128 = kc0
    (queries 0:128 of the block) and cols 128:384 = kc1 (queries 0:256).
    m45: cols 0:256 = kc4 (queries 0:256), cols 256:384 = kc5 (queries
    128:256). 1.0 keep, 0.0 drop."""
    p = np.arange(128)[:, None]          # key index within 128-chunk
    m01 = np.zeros((4, 128, 384), np.float32)
    m45 = np.zeros((4, 128, 384), np.float32)
    for qb in range(4):
        c_g = s_idx * 4 + qb

        def valid(kc, x):
            y = kc * 128 + p                      # window-local key pos (0..767)
            jg = c_g * 256 - 256 + y              # global key index
            ok = (y >= x) & (y <= x + 2 * WIN) & (jg >= 0) & (jg < S)
            return ok.astype(np.float32)

        m01[qb, :, 0:128] = valid(0, np.arange(128)[None, :])
        m01[qb, :, 128:384] = valid(1, np.arange(256)[None, :])
        m45[qb, :, 0:256] = valid(4, np.arange(256)[None, :])
        m45[qb, :, 256:384] = valid(5, np.arange(128, 256)[None, :])
    return m01, m45


_built = {}


def _get_nc():
    if "nc" not in _built:
        _built["nc"] = build()
    return _built["nc"]


def _bf16(x):
    import ml_dtypes
    return np.ascontiguousarray(np.asarray(x, np.float32).astype(ml_dtypes.bfloat16))


def make_in_maps(x, ln1_g, ln1_b, c_attn_w, c_attn_b, c_proj_w, c_proj_b,
                 ln2_g, ln2_b, fc_w, fc_b, proj2_w, proj2_b, w):
    assert int(w) == WIN
    f64 = np.float64
    w1 = (np.asarray(ln1_g, f64)[:, None] * np.asarray(c_attn_w, f64))
    bqkv = (np.asarray(ln1_b, f64) @ np.asarray(c_attn_w, f64)
            + np.asarray(c_attn_b, f64)).copy()
    w1[:, :E] *= 1.0 / np.sqrt(D)
    bqkv[:E] *= 1.0 / np.sqrt(D)
    w3 = (np.asarray(ln2_g, f64)[:, None] * np.asarray(fc_w, f64))
    b3 = np.asarray(ln2_b, f64) @ np.asarray(fc_w, f64) + np.asarray(fc_b, f64)

    common = {
        "w1": _bf16(w1),
        "b1": np.ascontiguousarray(
            np.asarray(bqkv[:2 * E], np.float32).reshape(12, 128).T),
        "bv": _bf16(bqkv[None, 2 * E:]),
        "wp": _bf16(c_proj_w),
        "bp": np.ascontiguousarray(
            np.asarray(c_proj_b, np.float32).reshape(KC, 128).T),
        "w3": _bf16(w3),
        "b3": np.ascontiguousarray(np.asarray(b3, np.float32).reshape(24, 128).T),
        "w4": _bf16(proj2_w),
        "b4": np.ascontiguousarray(
            np.asarray(proj2_b, np.float32).reshape(KC, 128).T),
        "ones": _bf16(np.ones((128, 128), np.float32)),
        "ones_f": np.ones((1, 128), np.float32),
    }
    masks = [_build_masks(s) for s in range(NSEQ)]
    x = np.asarray(x, np.float32)
    in_maps = []
    for ci in range(8):
        b, s = divmod(ci, NSEQ)
        xt = np.zeros((E, EXT), np.float32)
        lo = s * CHUNK - WIN
        hi = s * CHUNK + CHUNK + WIN
        slo, shi = max(lo, 0), min(hi, S)
        xt[:, slo - lo:shi - lo] = x[b, slo:shi, :].T
        m01, m45 = masks[s]
        in_maps.append(dict(common, xT=_bf16(xt), m01=_bf16(m01), m45=_bf16(m45)))
    return in_maps


def assemble(results):
    out = np.empty((B, S, E), np.float32)
    for ci in range(8):
        b, s = divmod(ci, NSEQ)
        out[b, s * CHUNK:(s + 1) * CHUNK, :] = np.asarray(
            results[ci]["out"], np.float32).T
    return out


def kernel(**inputs):
    in_maps = make_in_maps(**inputs)
    nc = _get_nc()
    res = run_bass_kernel_spmd(nc, in_maps, core_ids=list(range(8)))
    return assemble(res.results)
